# revision 10
# baseline (speedup 1.0000x reference)
"""Cross-attention Trainium2 kernel, v2.

Sharding: 8 cores = 2 batches x 4 head-groups (4 heads each).  Each core
computes a full (N, DIM) partial using its head-group's weight slices; the
host sums the 4 head-group partials per batch.

v2 changes vs baseline:
  - bf16 inputs converted on host; DMA lands directly in persistent SBUF
    (no fp32 staging, no on-chip casts).
  - Score matmuls row-tiled by m-chunk parity: KT for even m-chunks lives in
    partitions 0-63, odd in 64-127 (QT duplicated to both halves via
    SBUF->SBUF DMA).  The even/odd pair hits different PE row groups and
    different PSUM banks, so the array runs them concurrently.
  - exp split across ACT (table exp) and DVE (one-op Schraudolph: bf16 bit
    pattern = i16(A*s + B), truncation bias cancels in the softmax
    normalization).
  - reciprocal_approx_fast instead of full-precision reciprocal; normalize
    multiply on GPSIMD.

Device layout per core (everything transposed; no on-chip transposes):
    QT[c, n] = sum_k Wq[k, c] * xT[k, n]     (duplicated to both row halves)
    KT[c, m] = likewise from ctxT            (parity-split rows)
    V[m, c]  = sum_k ctxT[k, m] * Wv[k, c]   (+ ones col per head)
    ST[m, n] = sum_d KT[h d, m] QT[h d, n]   (even/odd mc pairs concurrent)
    PT[m, n] = exp(ST * scale)               (ACT exp or DVE Schraudolph)
    OT'[e,n] = sum_m V'[m, e] PT[m, n]       (e<64: out^T, e=64: denom)
    OTn      = OT' * approx(1/denom)         (DVE recip + gpsimd bcast/mul)
    out[n,c] = sum_hd OTn[hd, n] Wo[hd, c]
"""

import sys

sys.path.insert(0, "/opt/trn_rl_repo")

import numpy as np
import ml_dtypes

import concourse.bass as bass
import concourse.mybir as mybir
import concourse.tile as tile
from concourse import bacc
from concourse.bass_utils import run_bass_kernel_spmd

# Problem constants (hardcoded per harness contract).
B, N, M, DIM = 2, 2048, 2048, 1024
H_TOTAL, D = 16, 64
H = 4                      # local heads per core
HG = H_TOTAL // H          # 4 head groups
C_LOC = H * D              # 256 local projection width
SCALE = D ** -0.5
N_CORES = 8

KC = DIM // 128            # 8 contraction chunks
NB = N // 512              # 4 n blocks
MC = M // 128              # 16 m chunks
CB = DIM // 512            # 2 out col blocks

F32 = mybir.dt.float32
BF16 = mybir.dt.bfloat16
I16 = mybir.dt.int16

# Schraudolph exp in the bf16 bit domain: bits = trunc(A*s + B).
SCH_A = float((2.0 ** 7 / np.log(2.0)) * SCALE)
SCH_B = float(2.0 ** 7 * 126.946)


def dve_exp_tile(mg, i, j):
    """Which exp tiles go to the DVE (Schraudolph) vs ACT (exact).

    Strict 2/2 split per mg so neither engine's serial exp time sets the
    per-mg cadence; the chosen pair alternates with mg to decorrelate the
    Schraudolph error pattern."""
    return (i + j + mg) % 2 == 0             # 50% on DVE, 2+2 each mg


def build_program():
    nc = bacc.Bacc("TRN2", target_bir_lowering=False, debug=False)

    xt = nc.dram_tensor("xt", [DIM, N], BF16, kind="ExternalInput")
    ctxt = nc.dram_tensor("ctxt", [DIM, M], BF16, kind="ExternalInput")
    wq = nc.dram_tensor("wq", [DIM, C_LOC], BF16, kind="ExternalInput")
    wk = nc.dram_tensor("wk", [DIM, C_LOC], BF16, kind="ExternalInput")
    wv = nc.dram_tensor("wv", [DIM, C_LOC], BF16, kind="ExternalInput")
    wo = nc.dram_tensor("wo", [C_LOC, DIM], BF16, kind="ExternalInput")
    out = nc.dram_tensor("out", [N, DIM], F32, kind="ExternalOutput")

    with tile.TileContext(nc) as tc:
        with (
            tc.tile_pool(name="persist", bufs=1) as persist,
            tc.tile_pool(name="stg", bufs=4) as stg,
            tc.tile_pool(name="pt", bufs=8) as ptp,
            tc.tile_pool(name="bc", bufs=3) as bcp,
            tc.tile_pool(name="rcp", bufs=3) as rcp,
            tc.tile_pool(name="osb", bufs=3) as osb,
            tc.tile_pool(name="ps_proj", bufs=2, space="PSUM") as psp,
            tc.tile_pool(name="ps_s", bufs=4, space="PSUM") as pss,
            tc.tile_pool(name="ps_o", bufs=2, space="PSUM") as pso,
        ):
            # ---- persistent SBUF tensors (DMA lands here directly) ----
            xbf = persist.tile([128, KC, N], BF16)          # xT, k-chunked
            cbf = persist.tile([128, KC, M], BF16)          # ctxT, k-chunked
            wqbf = persist.tile([128, KC, C_LOC], BF16)
            wkbf = persist.tile([128, KC, C_LOC], BF16)
            wvbf = persist.tile([128, KC, C_LOC], BF16)
            wobf = persist.tile([128, 2, DIM], BF16)        # hd-chunked
            # QT duplicated to both row halves: [row=(64p+d), hp, j, n]
            qtbf = persist.tile([128, 2, 2, N], BF16)
            # KT parity-split: [row=(64*(mc%2)+d), hp, j, mc//2, 128]
            ktbf = persist.tile([128, 2, 2, MC // 2, 128], BF16)
            vpbf = persist.tile([128, MC, H * 65], BF16)    # V' with ones col
            otnbf = persist.tile([128, 2, N], BF16)         # normalized out^T

            # ---- weights: DMA direct, per-kc chunks so kt_proj starts early
            for w_dram, w_sb in ((wk, wkbf), (wv, wvbf), (wq, wqbf)):
                wv_r = w_dram[:].rearrange("(a p) c -> p a c", p=128)
                for kc in range(KC):
                    nc.gpsimd.dma_start(w_sb[:, kc, :], wv_r[:, kc, :])
            nc.gpsimd.dma_start(
                wobf[:], wo[:].rearrange("(a p) c -> p a c", p=128))

            # ---- emission helpers ----
            def ctx_block(nbm):
                mlo, mhi = nbm * 512, (nbm + 1) * 512
                for kc in range(KC):
                    nc.sync.dma_start(cbf[:, kc, mlo:mhi],
                                      ctxt[kc * 128:(kc + 1) * 128, mlo:mhi])

            def load_x(nb):
                nlo, nhi = nb * 512, (nb + 1) * 512
                for kc in range(KC):
                    nc.gpsimd.dma_start(xbf[:, kc, nlo:nhi],
                                        xt[kc * 128:(kc + 1) * 128, nlo:nhi])

            def kt_proj(nbm):
                # m block nbm covers mc = 4*nbm .. 4*nbm+3 -> slots 2nbm, 2nbm+1
                mlo, mhi = nbm * 512, (nbm + 1) * 512
                slo = 2 * nbm
                for hp in range(2):
                    ps = psp.tile([128, 512], F32, tag="proj", name=f"ktp{nbm}_{hp}")
                    for kc in range(KC):
                        nc.tensor.matmul(
                            ps[:],
                            wkbf[:, kc, hp * 128:(hp + 1) * 128],
                            cbf[:, kc, mlo:mhi],
                            start=(kc == 0),
                            stop=(kc == KC - 1),
                        )
                    s = stg.tile([128, 512], BF16, tag="stg", name=f"kts{nbm}_{hp}")
                    nc.scalar.activation(s[:], ps[:],
                                         mybir.ActivationFunctionType.Copy)
                    sv = s[:].rearrange("p (a c) -> p a c", c=128)  # a = local mc
                    for j in range(2):
                        for par in range(2):
                            # mcs with parity par -> rows 64*par..64*par+63
                            nc.sync.dma_start(
                                ktbf[64 * par:64 * par + 64, hp, j,
                                     slo:slo + 2, :],
                                sv[j * 64:(j + 1) * 64, par::2, :],
                            )

            def v_proj(nbm, copy_eng):
                for mc in range(nbm * 4, nbm * 4 + 4):
                    ps = psp.tile([128, C_LOC], F32, tag="proj", name=f"vp{mc}")
                    for kc in range(KC):
                        nc.tensor.matmul(
                            ps[:],
                            cbf[:, kc, mc * 128:(mc + 1) * 128],
                            wvbf[:, kc, :],
                            start=(kc == 0),
                            stop=(kc == KC - 1),
                        )
                    vslc = vpbf[:, mc, :].rearrange("p (h e) -> p h e", h=H)
                    eng = nc.vector if copy_eng == 0 else nc.scalar
                    if copy_eng == 0:
                        eng.tensor_copy(
                            vslc[:, :, 0:64],
                            ps[:].rearrange("p (h e) -> p h e", h=H))
                    else:
                        eng.activation(
                            vslc[:, :, 0:64],
                            ps[:].rearrange("p (h e) -> p h e", h=H),
                            mybir.ActivationFunctionType.Copy)
                    copy_eng ^= 1
                    nc.vector.memset(vslc[:, :, 64:65], 1.0)

            def qt_proj_hp(nb, hp):
                nlo, nhi = nb * 512, (nb + 1) * 512
                ps = psp.tile([128, 512], F32, tag="proj", name=f"qtp{nb}_{hp}")
                for kc in range(KC):
                    nc.tensor.matmul(
                        ps[:],
                        wqbf[:, kc, hp * 128:(hp + 1) * 128],
                        xbf[:, kc, nlo:nhi],
                        start=(kc == 0),
                        stop=(kc == KC - 1),
                    )
                s = stg.tile([128, 512], BF16, tag="stg", name=f"qts{nb}_{hp}")
                nc.scalar.activation(s[:], ps[:],
                                     mybir.ActivationFunctionType.Copy)
                for j in range(2):
                    for par in range(2):
                        nc.sync.dma_start(
                            qtbf[64 * par:64 * par + 64, hp, j, nlo:nhi],
                            s[j * 64:(j + 1) * 64, :],
                        )

            def qt_proj(nb):
                for hp in range(2):
                    qt_proj_hp(nb, hp)

            def attn_mgs(nb, hp, po, mgs, fillers=()):
                # Software-pipelined: scores+exp for mg+1 are emitted before
                # the AV matmuls of mg, so exp has a full iteration of lead
                # time and the AV never waits on the exp semaphore.
                fillers = list(fillers)
                nlo, nhi = nb * 512, (nb + 1) * 512
                pts = {}

                def scores_exp(mg):
                    # per-(j,par) one-bank PSUM tiles: finer lifetimes let the
                    # 5-slot pool truly pipeline mg+1's scores ahead of exp(mg)
                    pt = [None, None]
                    for j in range(2):
                        pt[j] = ptp.tile([128, 2, 512], I16, tag="pt",
                                         name=f"pt{nb}_{hp}_{mg}_{j}")
                    for j in range(2):
                        ssp = [pss.tile([128, 512], F32, tag="s",
                                        name=f"ss{nb}_{hp}_{mg}_{j}_{pp}")
                               for pp in range(2)]
                        # scores: even/odd m-chunk pair -> different PE row
                        # groups AND different PSUM banks -> run concurrently
                        for par in range(2):
                            nc.tensor.matmul(
                                ssp[par][:],
                                ktbf[64 * par:64 * par + 64, hp, j, mg, :],
                                qtbf[64 * par:64 * par + 64, hp, j, nlo:nhi],
                                start=True,
                                stop=True,
                            )
                        # exp: split across ACT / DVE
                        for i in range(2):
                            if dve_exp_tile(mg, i, j):
                                nc.vector.tensor_scalar(
                                    pt[j][:, i, :], ssp[i][:],
                                    SCH_A, SCH_B,
                                    mybir.AluOpType.mult, mybir.AluOpType.add,
                                )
                            else:
                                nc.scalar.activation(
                                    pt[j][:, i, :].bitcast(BF16), ssp[i][:],
                                    mybir.ActivationFunctionType.Exp,
                                    scale=SCALE,
                                )
                    pts[mg] = pt

                def av(mg):
                    pt = pts.pop(mg)
                    for j in range(2):
                        h = hp * 2 + j
                        for i in range(2):
                            mc = mg * 2 + i
                            nc.tensor.matmul(
                                po[j][:],
                                vpbf[:, mc, h * 65:(h + 1) * 65],
                                pt[j][:, i, :].bitcast(BF16),
                                start=(mc == 0),
                                stop=(mc == MC - 1),
                            )

                mgs = list(mgs)
                scores_exp(mgs[0])
                for idx, mg in enumerate(mgs):
                    if idx + 1 < len(mgs):
                        scores_exp(mgs[idx + 1])
                    av(mg)
                    if fillers:
                        fillers.pop(0)()
                for f in fillers:
                    f()

            def normalize(nb, hp, po):
                nlo, nhi = nb * 512, (nb + 1) * 512
                pofs, rts, bcs = [], [], []
                for j in range(2):
                    pof = bcp.tile([65, 512], F32, tag="pof", name=f"pof{nb}_{hp}_{j}")
                    if j == 0:
                        nc.scalar.activation(pof[:], po[j][:],
                                             mybir.ActivationFunctionType.Copy)
                    else:
                        nc.vector.tensor_copy(pof[:], po[j][:])
                    pofs.append(pof)
                for j in range(2):
                    # Scatter the denominator row across 64 partitions via
                    # SBUF->SBUF DMA so the reciprocal is free-size-bound at 8
                    # elements instead of 512 (also lands it at base partition
                    # 0, which custom-DVE ops require), then gather back.
                    dsc = rcp.tile([64, 8], F32, tag="dsc", name=f"ds{nb}_{hp}_{j}")
                    nc.sync.dma_start(dsc[:], pofs[j][64:65, :])
                    dsr = rcp.tile([64, 8], F32, tag="dsr", name=f"dr{nb}_{hp}_{j}")
                    nc.vector.reciprocal_approx_fast(dsr[:], dsc[:])
                    rt = rcp.tile([1, 512], F32, tag="rcp", name=f"rt{nb}_{hp}_{j}")
                    nc.sync.dma_start(rt[:], dsr[:])
                    rts.append(rt)
                for j in range(2):
                    bc = bcp.tile([64, 512], F32, tag="bc", name=f"bc{nb}_{hp}_{j}")
                    nc.gpsimd.partition_broadcast(bc[:], rts[j][:])
                    bcs.append(bc)
                for j in range(2):
                    nc.vector.tensor_mul(
                        otnbf[j * 64:(j + 1) * 64, hp, nlo:nhi],
                        pofs[j][0:64, :],
                        bcs[j][:],
                    )

            def attn_hp(nb, hp, fillers=()):
                po = [pso.tile([65, 512], F32, tag="ot",
                               name=f"po{nb}_{hp}_{jj}") for jj in range(2)]
                attn_mgs(nb, hp, po, range(MC // 2), fillers)
                normalize(nb, hp, po)

            def final_proj_ncx(ncx):
                o = osb.tile([128, 1024], F32, tag="osb", name=f"o{ncx}")
                for cb in range(CB):
                    ps = psp.tile([128, 512], F32, tag="proj",
                                  name=f"fp{ncx}_{cb}")
                    for hp in range(2):
                        nc.tensor.matmul(
                            ps[:],
                            otnbf[:, hp, ncx * 128:(ncx + 1) * 128],
                            wobf[:, hp, cb * 512:(cb + 1) * 512],
                            start=(hp == 0),
                            stop=(hp == 1),
                        )
                    if cb == 0:
                        nc.vector.tensor_copy(o[:, 0:512], ps[:])
                    else:
                        nc.scalar.activation(
                            o[:, 512:1024], ps[:],
                            mybir.ActivationFunctionType.Copy)
                nc.gpsimd.dma_start(out[ncx * 128:(ncx + 1) * 128, :], o[:])

            def final_proj(nb):
                for ncx in range(nb * 4, nb * 4 + 4):
                    final_proj_ncx(ncx)

            # ---- emission: phase 1 interleaved with first attention ----
            ctx_block(0)
            load_x(0)
            kt_proj(0)
            v_proj(0, 0)
            qt_proj(0)
            po0 = [pso.tile([65, 512], F32, tag="ot", name=f"po0_0_{jj}")
                   for jj in range(2)]
            for nbm in range(1, NB):
                ctx_block(nbm)
                attn_mgs(0, 0, po0, range(2 * (nbm - 1), 2 * nbm))
                kt_proj(nbm)
                v_proj(nbm, nbm % 2)
            attn_mgs(0, 0, po0, range(2 * (NB - 1), 2 * NB))
            normalize(0, 0, po0)
            load_x(1)
            attn_hp(0, 1, fillers=[lambda hp=hp: qt_proj_hp(1, hp)
                                   for hp in range(2)])
            # ---- steady state ----
            for nb in range(1, NB):
                if nb + 1 < NB:
                    load_x(nb + 1)
                f0 = [lambda ncx=ncx: final_proj_ncx(ncx)
                      for ncx in range((nb - 1) * 4, (nb - 1) * 4 + 2)]
                attn_hp(nb, 0, f0)
                fillers = [lambda ncx=ncx: final_proj_ncx(ncx)
                           for ncx in range((nb - 1) * 4 + 2, (nb - 1) * 4 + 4)]
                if nb + 1 < NB:
                    fillers += [lambda hp=hp: qt_proj_hp(nb + 1, hp)
                                for hp in range(2)]
                attn_hp(nb, 1, fillers)
            final_proj(NB - 1)

    nc.compile()
    return nc


_PROGRAM = None


def _get_program():
    global _PROGRAM
    if _PROGRAM is None:
        _PROGRAM = build_program()
    return _PROGRAM


def _bf(a):
    return np.ascontiguousarray(a).astype(ml_dtypes.bfloat16)


def make_in_maps(x, context, Wq, Wkv, Wo):
    x = np.asarray(x, dtype=np.float32)
    context = np.asarray(context, dtype=np.float32)
    Wq = np.asarray(Wq, dtype=np.float32)
    Wkv = np.asarray(Wkv, dtype=np.float32)
    Wo = np.asarray(Wo, dtype=np.float32)
    in_maps = []
    for core in range(N_CORES):
        b, hg = divmod(core, HG)
        cs = hg * C_LOC
        in_maps.append({
            "xt": _bf(x[b].T),
            "ctxt": _bf(context[b].T),
            "wq": _bf(Wq[:, cs:cs + C_LOC]),
            "wk": _bf(Wkv[:, cs:cs + C_LOC]),
            "wv": _bf(Wkv[:, DIM + cs:DIM + cs + C_LOC]),
            "wo": _bf(Wo[cs:cs + C_LOC, :]),
        })
    return in_maps


def kernel(x, context, mask, Wq, Wkv, Wo, _trace=False):
    # mask is all-ones per the input spec; the softmax ignores it.
    nc = _get_program()
    in_maps = make_in_maps(x, context, Wq, Wkv, Wo)
    res = run_bass_kernel_spmd(nc, in_maps, list(range(N_CORES)), trace=_trace)
    out = np.zeros((B, N, DIM), dtype=np.float32)
    for core in range(N_CORES):
        b = core // HG
        out[b] += res.results[core]["out"]
    if _trace:
        kernel.last_exec_time_ns = res.exec_time_ns
        kernel.last_trace = res.instructions_and_trace
    return out


def _partial_numpy(im):
    """Numpy re-computation of one core's partial (for sim validation)."""
    xT = np.asarray(im["xt"], dtype=np.float32)
    cT = np.asarray(im["ctxt"], dtype=np.float32)
    q = xT.T @ np.asarray(im["wq"], dtype=np.float32)
    k = cT.T @ np.asarray(im["wk"], dtype=np.float32)
    v = cT.T @ np.asarray(im["wv"], dtype=np.float32)
    wo_ = np.asarray(im["wo"], dtype=np.float32)
    partial = np.zeros((N, DIM), dtype=np.float32)
    for h in range(H):
        qh, kh, vh = (a[:, h * D:(h + 1) * D] for a in (q, k, v))
        s = (qh @ kh.T) * SCALE
        p = np.exp(s - s.max(axis=-1, keepdims=True))
        p /= p.sum(axis=-1, keepdims=True)
        partial += (p @ vh) @ wo_[h * D:(h + 1) * D, :]
    return partial


if __name__ == "__main__":
    mode = sys.argv[1] if len(sys.argv) > 1 else "sim"
    rng = np.random.default_rng(0)
    x = rng.standard_normal((B, N, DIM)).astype(np.float32)
    ctx_in = rng.standard_normal((B, M, DIM)).astype(np.float32)
    s = DIM ** -0.5
    Wq_ = (rng.standard_normal((DIM, DIM)) * s).astype(np.float32)
    Wkv_ = (rng.standard_normal((DIM, 2 * DIM)) * s).astype(np.float32)
    Wo_ = (rng.standard_normal((DIM, DIM)) * s).astype(np.float32)
    in_maps = make_in_maps(x, ctx_in, Wq_, Wkv_, Wo_)

    if mode == "sim":
        from concourse.bass_interp import CoreSim
        nc = _get_program()
        sim = CoreSim(nc)
        im = in_maps[0]
        for k_, v_ in im.items():
            sim.tensor(k_)[:] = v_
        sim.simulate(check_with_hw=False)
        got = np.array(sim.tensor("out"))
        want = _partial_numpy(im)
        denom = np.abs(want).max()
        print("max abs err:", np.abs(got - want).max(),
              " rel:", np.abs(got - want).max() / denom)
    elif mode == "hw":
        nc = _get_program()
        res = run_bass_kernel_spmd(nc, in_maps, list(range(N_CORES)))
        for core in range(N_CORES):
            got = res.results[core]["out"]
            want = _partial_numpy(in_maps[core])
            err = np.abs(got - want).max() / np.abs(want).max()
            print(f"core {core}: rel err {err:.2e}")


# revision 11
# speedup vs baseline: 1.1758x; 1.1758x over previous
"""Cross-attention Trainium2 kernel, v2.

Sharding: 8 cores = 2 batches x 4 head-groups (4 heads each).  Each core
computes a full (N, DIM) partial using its head-group's weight slices; the
host sums the 4 head-group partials per batch.

v2 changes vs baseline:
  - bf16 inputs converted on host; DMA lands directly in persistent SBUF
    (no fp32 staging, no on-chip casts).
  - Score matmuls row-tiled by m-chunk parity: KT for even m-chunks lives in
    partitions 0-63, odd in 64-127 (QT duplicated to both halves via
    SBUF->SBUF DMA).  The even/odd pair hits different PE row groups and
    different PSUM banks, so the array runs them concurrently.
  - exp split across ACT (table exp) and DVE (one-op Schraudolph: bf16 bit
    pattern = i16(A*s + B), truncation bias cancels in the softmax
    normalization).
  - reciprocal_approx_fast instead of full-precision reciprocal; normalize
    multiply on GPSIMD.

Device layout per core (everything transposed; no on-chip transposes):
    QT[c, n] = sum_k Wq[k, c] * xT[k, n]     (duplicated to both row halves)
    KT[c, m] = likewise from ctxT            (parity-split rows)
    V[m, c]  = sum_k ctxT[k, m] * Wv[k, c]   (+ ones col per head)
    ST[m, n] = sum_d KT[h d, m] QT[h d, n]   (even/odd mc pairs concurrent)
    PT[m, n] = exp(ST * scale)               (ACT exp or DVE Schraudolph)
    OT'[e,n] = sum_m V'[m, e] PT[m, n]       (e<64: out^T, e=64: denom)
    OTn      = OT' * approx(1/denom)         (DVE recip + gpsimd bcast/mul)
    out[n,c] = sum_hd OTn[hd, n] Wo[hd, c]
"""

import sys

sys.path.insert(0, "/opt/trn_rl_repo")

import numpy as np
import ml_dtypes

import concourse.bass as bass
import concourse.mybir as mybir
import concourse.tile as tile
from concourse import bacc
from concourse.bass_utils import run_bass_kernel_spmd

# Problem constants (hardcoded per harness contract).
B, N, M, DIM = 2, 2048, 2048, 1024
H_TOTAL, D = 16, 64
H = 4                      # local heads per core
HG = H_TOTAL // H          # 4 head groups
C_LOC = H * D              # 256 local projection width
SCALE = D ** -0.5
N_CORES = 8

KC = DIM // 128            # 8 contraction chunks
NB = N // 512              # 4 n blocks
MC = M // 128              # 16 m chunks
CB = DIM // 512            # 2 out col blocks

F32 = mybir.dt.float32
BF16 = mybir.dt.bfloat16
I16 = mybir.dt.int16

# Schraudolph exp in the bf16 bit domain: bits = trunc(A*s + B).
SCH_A = float((2.0 ** 7 / np.log(2.0)) * SCALE)
SCH_B = float(2.0 ** 7 * 126.946)


def dve_exp_tile(mg, i, j):
    """Which exp tiles go to the DVE (Schraudolph) vs ACT (exact).

    Strict 2/2 split per mg so neither engine's serial exp time sets the
    per-mg cadence; the chosen pair alternates with mg to decorrelate the
    Schraudolph error pattern."""
    return (i + j + mg) % 2 == 0             # 50% on DVE, 2+2 each mg


def build_program():
    nc = bacc.Bacc("TRN2", target_bir_lowering=False, debug=False)

    xt = nc.dram_tensor("xt", [DIM, N], BF16, kind="ExternalInput")
    ctxt = nc.dram_tensor("ctxt", [DIM, M], BF16, kind="ExternalInput")
    wq = nc.dram_tensor("wq", [DIM, C_LOC], BF16, kind="ExternalInput")
    wk = nc.dram_tensor("wk", [DIM, C_LOC], BF16, kind="ExternalInput")
    wv = nc.dram_tensor("wv", [DIM, C_LOC], BF16, kind="ExternalInput")
    wo = nc.dram_tensor("wo", [C_LOC, DIM], BF16, kind="ExternalInput")
    out = nc.dram_tensor("out", [N, DIM], F32, kind="ExternalOutput")

    with tile.TileContext(nc) as tc:
        with (
            tc.tile_pool(name="persist", bufs=1) as persist,
            tc.tile_pool(name="stg", bufs=4) as stg,
            tc.tile_pool(name="pt", bufs=8) as ptp,
            tc.tile_pool(name="bc", bufs=3) as bcp,
            tc.tile_pool(name="rcp", bufs=3) as rcp,
            tc.tile_pool(name="osb", bufs=3) as osb,
            tc.tile_pool(name="ps_proj", bufs=2, space="PSUM") as psp,
            tc.tile_pool(name="ps_s", bufs=4, space="PSUM") as pss,
            tc.tile_pool(name="ps_o", bufs=2, space="PSUM") as pso,
        ):
            # ---- persistent SBUF tensors (DMA lands here directly) ----
            xbf = persist.tile([128, KC, N], BF16)          # xT, k-chunked
            cbf = persist.tile([128, KC, M], BF16)          # ctxT, k-chunked
            wqbf = persist.tile([128, KC, C_LOC], BF16)
            wkbf = persist.tile([128, KC, C_LOC], BF16)
            wvbf = persist.tile([128, KC, C_LOC], BF16)
            wobf = persist.tile([128, 2, DIM], BF16)        # hd-chunked
            # QT duplicated to both row halves: [row=(64p+d), hp, j, n]
            qtbf = persist.tile([128, 2, 2, N], BF16)
            # KT parity-split: [row=(64*(mc%2)+d), hp, j, mc//2, 128]
            ktbf = persist.tile([128, 2, 2, MC // 2, 128], BF16)
            vpbf = persist.tile([128, MC, H * 65], BF16)    # V' with ones col
            otnbf = persist.tile([128, 2, N], BF16)         # normalized out^T

            # ---- weights: DMA direct, per-kc chunks so kt_proj starts early
            for w_dram, w_sb in ((wk, wkbf), (wv, wvbf), (wq, wqbf)):
                wv_r = w_dram[:].rearrange("(a p) c -> p a c", p=128)
                for kc in range(KC):
                    nc.gpsimd.dma_start(w_sb[:, kc, :], wv_r[:, kc, :])
            nc.gpsimd.dma_start(
                wobf[:], wo[:].rearrange("(a p) c -> p a c", p=128))

            # ---- emission helpers ----
            def ctx_block(nbm):
                mlo, mhi = nbm * 512, (nbm + 1) * 512
                for kc in range(KC):
                    nc.sync.dma_start(cbf[:, kc, mlo:mhi],
                                      ctxt[kc * 128:(kc + 1) * 128, mlo:mhi])

            def load_x(nb):
                nlo, nhi = nb * 512, (nb + 1) * 512
                for kc in range(KC):
                    nc.gpsimd.dma_start(xbf[:, kc, nlo:nhi],
                                        xt[kc * 128:(kc + 1) * 128, nlo:nhi])

            def kt_proj(nbm):
                # m block nbm covers mc = 4*nbm .. 4*nbm+3 -> slots 2nbm, 2nbm+1
                mlo, mhi = nbm * 512, (nbm + 1) * 512
                slo = 2 * nbm
                for hp in range(2):
                    ps = psp.tile([128, 512], F32, tag="proj", name=f"ktp{nbm}_{hp}")
                    for kc in range(KC):
                        nc.tensor.matmul(
                            ps[:],
                            wkbf[:, kc, hp * 128:(hp + 1) * 128],
                            cbf[:, kc, mlo:mhi],
                            start=(kc == 0),
                            stop=(kc == KC - 1),
                        )
                    s = stg.tile([128, 512], BF16, tag="stg", name=f"kts{nbm}_{hp}")
                    nc.scalar.activation(s[:], ps[:],
                                         mybir.ActivationFunctionType.Copy)
                    sv = s[:].rearrange("p (a c) -> p a c", c=128)  # a = local mc
                    for j in range(2):
                        for par in range(2):
                            # mcs with parity par -> rows 64*par..64*par+63
                            nc.sync.dma_start(
                                ktbf[64 * par:64 * par + 64, hp, j,
                                     slo:slo + 2, :],
                                sv[j * 64:(j + 1) * 64, par::2, :],
                            )

            def v_proj(nbm, copy_eng):
                for mc in range(nbm * 4, nbm * 4 + 4):
                    ps = psp.tile([128, C_LOC], F32, tag="proj", name=f"vp{mc}")
                    for kc in range(KC):
                        nc.tensor.matmul(
                            ps[:],
                            cbf[:, kc, mc * 128:(mc + 1) * 128],
                            wvbf[:, kc, :],
                            start=(kc == 0),
                            stop=(kc == KC - 1),
                        )
                    vslc = vpbf[:, mc, :].rearrange("p (h e) -> p h e", h=H)
                    eng = nc.vector if copy_eng == 0 else nc.scalar
                    if copy_eng == 0:
                        eng.tensor_copy(
                            vslc[:, :, 0:64],
                            ps[:].rearrange("p (h e) -> p h e", h=H))
                    else:
                        eng.activation(
                            vslc[:, :, 0:64],
                            ps[:].rearrange("p (h e) -> p h e", h=H),
                            mybir.ActivationFunctionType.Copy)
                    copy_eng ^= 1
                    nc.vector.memset(vslc[:, :, 64:65], 1.0)

            def qt_proj_hp(nb, hp):
                nlo, nhi = nb * 512, (nb + 1) * 512
                ps = psp.tile([128, 512], F32, tag="proj", name=f"qtp{nb}_{hp}")
                for kc in range(KC):
                    nc.tensor.matmul(
                        ps[:],
                        wqbf[:, kc, hp * 128:(hp + 1) * 128],
                        xbf[:, kc, nlo:nhi],
                        start=(kc == 0),
                        stop=(kc == KC - 1),
                    )
                s = stg.tile([128, 512], BF16, tag="stg", name=f"qts{nb}_{hp}")
                nc.scalar.activation(s[:], ps[:],
                                     mybir.ActivationFunctionType.Copy)
                for j in range(2):
                    for par in range(2):
                        nc.sync.dma_start(
                            qtbf[64 * par:64 * par + 64, hp, j, nlo:nhi],
                            s[j * 64:(j + 1) * 64, :],
                        )

            def qt_proj(nb):
                for hp in range(2):
                    qt_proj_hp(nb, hp)

            def attn_mgs(nb, hp, po, mgs, fillers=()):
                # Software-pipelined: scores+exp for mg+1 are emitted before
                # the AV matmuls of mg, so exp has a full iteration of lead
                # time and the AV never waits on the exp semaphore.
                fillers = list(fillers)
                nlo, nhi = nb * 512, (nb + 1) * 512
                pts = {}

                def scores_exp(mg):
                    # per-(j,par) one-bank PSUM tiles: finer lifetimes let the
                    # 5-slot pool truly pipeline mg+1's scores ahead of exp(mg)
                    pt = [None, None]
                    for j in range(2):
                        pt[j] = ptp.tile([128, 2, 512], I16, tag="pt",
                                         name=f"pt{nb}_{hp}_{mg}_{j}")
                    for j in range(2):
                        ssp = [pss.tile([128, 512], F32, tag="s",
                                        name=f"ss{nb}_{hp}_{mg}_{j}_{pp}")
                               for pp in range(2)]
                        # scores: even/odd m-chunk pair -> different PE row
                        # groups AND different PSUM banks -> run concurrently
                        for par in range(2):
                            nc.tensor.matmul(
                                ssp[par][:],
                                ktbf[64 * par:64 * par + 64, hp, j, mg, :],
                                qtbf[64 * par:64 * par + 64, hp, j, nlo:nhi],
                                start=True,
                                stop=True,
                            )
                        # exp: split across ACT / DVE
                        for i in range(2):
                            if dve_exp_tile(mg, i, j):
                                nc.vector.tensor_scalar(
                                    pt[j][:, i, :], ssp[i][:],
                                    SCH_A, SCH_B,
                                    mybir.AluOpType.mult, mybir.AluOpType.add,
                                )
                            else:
                                nc.scalar.activation(
                                    pt[j][:, i, :].bitcast(BF16), ssp[i][:],
                                    mybir.ActivationFunctionType.Exp,
                                    scale=SCALE,
                                )
                    pts[mg] = pt

                def av(mg):
                    pt = pts.pop(mg)
                    for j in range(2):
                        h = hp * 2 + j
                        for i in range(2):
                            mc = mg * 2 + i
                            nc.tensor.matmul(
                                po[j][:],
                                vpbf[:, mc, h * 65:(h + 1) * 65],
                                pt[j][:, i, :].bitcast(BF16),
                                start=(mc == 0),
                                stop=(mc == MC - 1),
                            )

                mgs = list(mgs)
                scores_exp(mgs[0])
                for idx, mg in enumerate(mgs):
                    if idx + 1 < len(mgs):
                        scores_exp(mgs[idx + 1])
                    av(mg)
                    if fillers:
                        fillers.pop(0)()
                for f in fillers:
                    f()

            def normalize(nb, hp, po):
                nlo, nhi = nb * 512, (nb + 1) * 512
                pofs, rts, bcs = [], [], []
                for j in range(2):
                    pof = bcp.tile([65, 512], F32, tag="pof", name=f"pof{nb}_{hp}_{j}")
                    if j == 0:
                        nc.scalar.activation(pof[:], po[j][:],
                                             mybir.ActivationFunctionType.Copy)
                    else:
                        nc.vector.tensor_copy(pof[:], po[j][:])
                    pofs.append(pof)
                for j in range(2):
                    # Scatter the denominator row across 64 partitions via
                    # SBUF->SBUF DMA so the reciprocal is free-size-bound at 8
                    # elements instead of 512 (also lands it at base partition
                    # 0, which custom-DVE ops require), then gather back.
                    dsc = rcp.tile([64, 8], F32, tag="dsc", name=f"ds{nb}_{hp}_{j}")
                    nc.sync.dma_start(dsc[:], pofs[j][64:65, :])
                    dsr = rcp.tile([64, 8], F32, tag="dsr", name=f"dr{nb}_{hp}_{j}")
                    nc.vector.reciprocal_approx_fast(dsr[:], dsc[:])
                    rt = rcp.tile([1, 512], F32, tag="rcp", name=f"rt{nb}_{hp}_{j}")
                    nc.sync.dma_start(rt[:], dsr[:])
                    rts.append(rt)
                for j in range(2):
                    bc = bcp.tile([64, 512], F32, tag="bc", name=f"bc{nb}_{hp}_{j}")
                    nc.gpsimd.partition_broadcast(bc[:], rts[j][:])
                    bcs.append(bc)
                for j in range(2):
                    nc.vector.tensor_mul(
                        otnbf[j * 64:(j + 1) * 64, hp, nlo:nhi],
                        pofs[j][0:64, :],
                        bcs[j][:],
                    )

            def attn_hp(nb, hp, fillers=()):
                po = [pso.tile([65, 512], F32, tag="ot",
                               name=f"po{nb}_{hp}_{jj}") for jj in range(2)]
                attn_mgs(nb, hp, po, range(MC // 2), fillers)
                normalize(nb, hp, po)

            def final_proj_ncx(ncx):
                o = osb.tile([128, 1024], F32, tag="osb", name=f"o{ncx}")
                for cb in range(CB):
                    ps = psp.tile([128, 512], F32, tag="proj",
                                  name=f"fp{ncx}_{cb}")
                    for hp in range(2):
                        nc.tensor.matmul(
                            ps[:],
                            otnbf[:, hp, ncx * 128:(ncx + 1) * 128],
                            wobf[:, hp, cb * 512:(cb + 1) * 512],
                            start=(hp == 0),
                            stop=(hp == 1),
                        )
                    if cb == 0:
                        nc.vector.tensor_copy(o[:, 0:512], ps[:])
                    else:
                        nc.scalar.activation(
                            o[:, 512:1024], ps[:],
                            mybir.ActivationFunctionType.Copy)
                nc.gpsimd.dma_start(out[ncx * 128:(ncx + 1) * 128, :], o[:])

            def final_proj(nb):
                for ncx in range(nb * 4, nb * 4 + 4):
                    final_proj_ncx(ncx)

            # ---- emission: phase 1 interleaved with first attention ----
            ctx_block(0)
            load_x(0)
            kt_proj(0)
            v_proj(0, 0)
            qt_proj(0)
            po0 = [pso.tile([65, 512], F32, tag="ot", name=f"po0_0_{jj}")
                   for jj in range(2)]
            for nbm in range(1, NB):
                ctx_block(nbm)
                attn_mgs(0, 0, po0, range(2 * (nbm - 1), 2 * nbm))
                kt_proj(nbm)
                v_proj(nbm, nbm % 2)
            attn_mgs(0, 0, po0, range(2 * (NB - 1), 2 * NB))
            normalize(0, 0, po0)
            load_x(1)
            attn_hp(0, 1, fillers=[lambda hp=hp: qt_proj_hp(1, hp)
                                   for hp in range(2)])
            # ---- steady state ----
            for nb in range(1, NB):
                if nb + 1 < NB:
                    load_x(nb + 1)
                attn_hp(nb, 0)
                fillers = [lambda ncx=ncx: final_proj_ncx(ncx)
                           for ncx in range((nb - 1) * 4, (nb - 1) * 4 + 4)]
                if nb + 1 < NB:
                    fillers += [lambda hp=hp: qt_proj_hp(nb + 1, hp)
                                for hp in range(2)]
                attn_hp(nb, 1, fillers)
            final_proj(NB - 1)

    nc.compile()
    return nc


_PROGRAM = None


def _get_program():
    global _PROGRAM
    if _PROGRAM is None:
        _PROGRAM = build_program()
    return _PROGRAM


def _bf(a):
    return np.ascontiguousarray(a).astype(ml_dtypes.bfloat16)


def make_in_maps(x, context, Wq, Wkv, Wo):
    x = np.asarray(x, dtype=np.float32)
    context = np.asarray(context, dtype=np.float32)
    Wq = np.asarray(Wq, dtype=np.float32)
    Wkv = np.asarray(Wkv, dtype=np.float32)
    Wo = np.asarray(Wo, dtype=np.float32)
    in_maps = []
    for core in range(N_CORES):
        b, hg = divmod(core, HG)
        cs = hg * C_LOC
        in_maps.append({
            "xt": _bf(x[b].T),
            "ctxt": _bf(context[b].T),
            "wq": _bf(Wq[:, cs:cs + C_LOC]),
            "wk": _bf(Wkv[:, cs:cs + C_LOC]),
            "wv": _bf(Wkv[:, DIM + cs:DIM + cs + C_LOC]),
            "wo": _bf(Wo[cs:cs + C_LOC, :]),
        })
    return in_maps


def kernel(x, context, mask, Wq, Wkv, Wo, _trace=False):
    # mask is all-ones per the input spec; the softmax ignores it.
    nc = _get_program()
    in_maps = make_in_maps(x, context, Wq, Wkv, Wo)
    res = run_bass_kernel_spmd(nc, in_maps, list(range(N_CORES)), trace=_trace)
    out = np.zeros((B, N, DIM), dtype=np.float32)
    for core in range(N_CORES):
        b = core // HG
        out[b] += res.results[core]["out"]
    if _trace:
        kernel.last_exec_time_ns = res.exec_time_ns
        kernel.last_trace = res.instructions_and_trace
    return out


def _partial_numpy(im):
    """Numpy re-computation of one core's partial (for sim validation)."""
    xT = np.asarray(im["xt"], dtype=np.float32)
    cT = np.asarray(im["ctxt"], dtype=np.float32)
    q = xT.T @ np.asarray(im["wq"], dtype=np.float32)
    k = cT.T @ np.asarray(im["wk"], dtype=np.float32)
    v = cT.T @ np.asarray(im["wv"], dtype=np.float32)
    wo_ = np.asarray(im["wo"], dtype=np.float32)
    partial = np.zeros((N, DIM), dtype=np.float32)
    for h in range(H):
        qh, kh, vh = (a[:, h * D:(h + 1) * D] for a in (q, k, v))
        s = (qh @ kh.T) * SCALE
        p = np.exp(s - s.max(axis=-1, keepdims=True))
        p /= p.sum(axis=-1, keepdims=True)
        partial += (p @ vh) @ wo_[h * D:(h + 1) * D, :]
    return partial


if __name__ == "__main__":
    mode = sys.argv[1] if len(sys.argv) > 1 else "sim"
    rng = np.random.default_rng(0)
    x = rng.standard_normal((B, N, DIM)).astype(np.float32)
    ctx_in = rng.standard_normal((B, M, DIM)).astype(np.float32)
    s = DIM ** -0.5
    Wq_ = (rng.standard_normal((DIM, DIM)) * s).astype(np.float32)
    Wkv_ = (rng.standard_normal((DIM, 2 * DIM)) * s).astype(np.float32)
    Wo_ = (rng.standard_normal((DIM, DIM)) * s).astype(np.float32)
    in_maps = make_in_maps(x, ctx_in, Wq_, Wkv_, Wo_)

    if mode == "sim":
        from concourse.bass_interp import CoreSim
        nc = _get_program()
        sim = CoreSim(nc)
        im = in_maps[0]
        for k_, v_ in im.items():
            sim.tensor(k_)[:] = v_
        sim.simulate(check_with_hw=False)
        got = np.array(sim.tensor("out"))
        want = _partial_numpy(im)
        denom = np.abs(want).max()
        print("max abs err:", np.abs(got - want).max(),
              " rel:", np.abs(got - want).max() / denom)
    elif mode == "hw":
        nc = _get_program()
        res = run_bass_kernel_spmd(nc, in_maps, list(range(N_CORES)))
        for core in range(N_CORES):
            got = res.results[core]["out"]
            want = _partial_numpy(in_maps[core])
            err = np.abs(got - want).max() / np.abs(want).max()
            print(f"core {core}: rel err {err:.2e}")
